# revision 25
# baseline (speedup 1.0000x reference)
"""BarrierNet (MLP 4->512->{128,128}->{2,2} + closed-form QP) on 8 Trainium2 cores.

Data-parallel: batch 262144 sharded 8 x 32768; weights replicated.

v2 design (cost-model driven):
- L1 runs as fp8e4m3 DoubleRow matmuls (0.5 cyc/row): the 4(+bias) input
  features are expanded host-side into a 4-term hi/lo fp8 split (20
  contraction items packed as K=10 x 2 planes), so L1 keeps ~bf16 accuracy
  at half the f32r streaming cost.
- L2 contracts the 512 h1 neurons as fp8 DoubleRow (2 matmuls of K=256 per
  output half) plus a 1-pair DoubleRow bias matmul (hi/lo planes), so the
  psum holds relu-ready pre-activations and the vector pass is a pure
  scale+relu+cast.
- L3 is transposed: lhsT = h2 (stationary), rhs = tiny W3 column block, so
  outputs land sample-major [128, 4] in psum at ~7ns/matmul and the QP phase
  needs no transposes, DRAM bounces, or staging matmuls.
- The three psum->sbuf passes per tile (h1 relu x2, h2 relu) are split
  across ACT / Pool / DVE to balance engine time; the QP chain runs wide
  [128, 256] with scalar_tensor_tensor fusion, ACT Square/Sin/Abs offload,
  and sample-major psum results copied straight into SBUF.
"""
import numpy as np
import ml_dtypes
from contextlib import ExitStack

import concourse.bass as bass
from concourse import bacc as bacc_mod
import concourse.tile as tile
from concourse import mybir
from concourse.bass_utils import run_bass_kernel_spmd

F32 = mybir.dt.float32
F32R = mybir.dt.float32r
F8 = mybir.dt.float8e4
E4 = ml_dtypes.float8_e4m3
AF = mybir.ActivationFunctionType
OP = mybir.AluOpType
DRM = mybir.MatmulPerfMode.DoubleRow

MAGIC = float(np.float32(1.5 * 2 ** 23))
INV2PI = float(np.float32(1.0 / (2 * np.pi)))
TWOPI = float(np.float32(2 * np.pi))
HALFPI = float(np.float32(np.pi / 2))
GG_EPS = float(np.float32(1e-12 / 1600.0))

N_CORES = 8
NB = 262144
B = NB // N_CORES      # 32768 per core
T = B // 512           # 64 tiles of 512 samples
F = B // 128           # 256 samples per partition
NSP = T // 32          # 2 sp groups (32 tiles / 128 f's each)
CB = 448               # reluB col split: Pool [0:CB], ACT [CB:512]

_CACHE = {}


def _emit(nc, tc, ctx, aps, B):
    (xdr, xq, w1dr, w2dr, b2dr, b2f32, ones8, w3d, onescol, b3row,
     u_out) = aps
    T = B // 512
    NSP = T // 32

    const = ctx.enter_context(tc.tile_pool(name="const", bufs=1))
    hp = ctx.enter_context(tc.tile_pool(name="hp", bufs=1))
    qp = ctx.enter_context(tc.tile_pool(name="qp", bufs=1))
    pairs = ctx.enter_context(tc.tile_pool(name="pairs", bufs=1, space="PSUM"))
    p3p = ctx.enter_context(tc.tile_pool(name="p3p", bufs=1, space="PSUM"))

    # ---- constant staging (order = DMA issue order) ----
    # xdr split in 4 so tile 0 can start after the first quarter lands
    sb_xdr = const.tile([10, B // 512, 2, 512], F8, name="sb_xdr",
                        tag="sb_xdr")
    TT = B // 512
    XCH = [0, 4, 16, TT // 2, TT]
    nc.sync.dma_start(sb_xdr[:, 0:4], xdr[:, 0:4])
    sb_w1 = const.tile([10, 2, 4, 128], F8, name="sb_w1", tag="sb_w1")
    nc.sync.dma_start(sb_w1[:], w1dr[:])
    nc.sync.dma_start(sb_xdr[:, XCH[1]:XCH[2]], xdr[:, XCH[1]:XCH[2]])
    sb_w2 = const.tile([128, 2, 2, 2, 128], F8, name="sb_w2", tag="sb_w2")
    nc.sync.dma_start(sb_w2[:], w2dr[:])
    sb_b2 = const.tile([1, 2, 2, 128], F8, name="sb_b2", tag="sb_b2")
    nc.sync.dma_start(sb_b2[:], b2dr[:])
    sb_ones8 = const.tile([1, 2, 512], F8, name="sb_ones8", tag="sb_ones8")
    nc.sync.dma_start(sb_ones8[:], ones8[:])
    sb_w3 = const.tile([128, 2, 4], F32R, name="sb_w3", tag="sb_w3")
    nc.sync.dma_start(sb_w3[:], w3d[:])
    sb_onescol = const.tile([1, 128], F32R, name="sb_onescol", tag="sb_onescol")
    nc.sync.dma_start(sb_onescol[:], onescol[:])
    sb_b3row = const.tile([1, 4], F32R, name="sb_b3row", tag="sb_b3row")
    nc.sync.dma_start(sb_b3row[:], b3row[:])
    sb_xq = const.tile([128, 4, B // 128], F32, name="sb_xq", tag="sb_xq")
    nc.sync.dma_start(sb_xq[:], xq[:])
    for k in range(2, 4):
        nc.sync.dma_start(sb_xdr[:, XCH[k]:XCH[k + 1]],
                          xdr[:, XCH[k]:XCH[k + 1]])

    stage = const.tile([128, NSP, 32, 4, 4], F32, name="stage", tag="stage")
    u_sb = const.tile([128, NSP, 32, 4, 2], F32, name="u_sb", tag="u_sb")

    bias05 = const.tile([128, 1], F32, name="bias05", tag="bias05")
    nc.gpsimd.memset(bias05[:], 0.5)
    bias25 = const.tile([128, 1], F32, name="bias25", tag="bias25")
    nc.gpsimd.memset(bias25[:], 2.5)
    biasHP = const.tile([128, 1], F32, name="biasHP", tag="biasHP")
    nc.gpsimd.memset(biasHP[:], HALFPI)
    # warm every ACT table during the input-DMA shadow so no
    # LoadActFuncSet lands mid-pipeline
    warm = const.tile([128, 1], F32, name="warm", tag="warm")
    for fn in (AF.Relu, AF.Copy, AF.Abs, AF.Sin, AF.Sigmoid):
        nc.scalar.activation(warm[:], bias05[:], fn)

    # ---- QP program ----
    PX = sb_xq[:, 0, :].rearrange("p (s g j) -> p s g j", s=NSP, g=32, j=4)
    PY = sb_xq[:, 1, :].rearrange("p (s g j) -> p s g j", s=NSP, g=32, j=4)
    TH = sb_xq[:, 2, :].rearrange("p (s g j) -> p s g j", s=NSP, g=32, j=4)
    VV = sb_xq[:, 3, :].rearrange("p (s g j) -> p s g j", s=NSP, g=32, j=4)

    def gt(name):
        return qp.tile([128, NSP, 32, 4], F32, name=name, tag=name)

    g = {}

    def geo_thunks():
        # each thunk emits ONE instruction; order respects deps.
        # Pool (gpsimd) does all SBUF->SBUF tensor work (it cannot touch
        # PSUM), ACT does Sin/Abs, DVE only the reciprocal.
        def dve(name, fn):
            def run():
                g[name] = gt(name)
                fn(g[name][:])
            return run

        def mk_ts(name, src, s1, op0, s2=None, op1=None):
            def emit(o):
                a = src() if callable(src) else src
                if op1 is None:
                    nc.gpsimd.tensor_scalar(o, a, s1, None, op0=op0)
                else:
                    nc.gpsimd.tensor_scalar(o, a, s1, s2, op0=op0, op1=op1)
            return dve(name, emit)

        def mk_tt(name, a, b, op):
            return dve(name, lambda o: nc.gpsimd.tensor_tensor(
                o, a(), b(), op=op))

        def mk_act(name, a, func, scale=1.0, bias=0.0):
            return dve(name, lambda o: nc.scalar.activation(
                o, a(), func, scale=scale, bias=bias))

        G = lambda n: (lambda: g[n][:])
        return [
            mk_ts("dxp", lambda: PX, 1.0, OP.add),
            mk_ts("dyp", lambda: PY, 0.5, OP.add),
            mk_ts("vp", lambda: VV, 2.5, OP.add),
            mk_ts("f1", lambda: TH, INV2PI, OP.mult, MAGIC, OP.add),
            mk_ts("f2", G("f1"), MAGIC, OP.subtract, TWOPI, OP.mult),
            mk_tt("thr", lambda: TH, G("f2"), OP.subtract),
            mk_act("st", G("thr"), AF.Sin),
            mk_act("ab", G("thr"), AF.Abs),
            mk_act("ct", G("ab"), AF.Sin, scale=-1.0, bias=biasHP[:]),
            mk_tt("vst2", G("vp"), G("st"), OP.mult),
            mk_tt("vct2", G("vp"), G("ct"), OP.mult),
            mk_tt("ga", G("dxp"), G("vst2"), OP.mult),
            mk_tt("gb", G("dyp"), G("vct2"), OP.mult),
            mk_tt("G1q", G("ga"), G("gb"), OP.subtract),
            mk_tt("gc", G("dxp"), G("vct2"), OP.mult),
            mk_tt("gd", G("dyp"), G("vst2"), OP.mult),
            mk_tt("S", G("gc"), G("gd"), OP.add),
            mk_tt("ge", G("dxp"), G("ct"), OP.mult),
            mk_tt("gf", G("dyp"), G("st"), OP.mult),
            mk_tt("G2q", G("ge"), G("gf"), OP.add),
            mk_tt("q1", G("G1q"), G("G1q"), OP.mult),
            mk_tt("q2t", G("G2q"), G("G2q"), OP.mult),
            mk_ts("q2", G("q2t"), 0.25, OP.mult),
            mk_tt("q3", G("q1"), G("q2"), OP.add),
            mk_ts("ggc", G("q3"), GG_EPS, OP.max),
            mk_tt("dx2", G("dxp"), G("dxp"), OP.mult),
            mk_tt("dy2", G("dyp"), G("dyp"), OP.mult),
            mk_tt("barq", G("dx2"), G("dy2"), OP.add),
            mk_tt("v2q", G("vp"), G("vp"), OP.mult),
            dve("rec", lambda o: nc.vector.reciprocal(o, g["ggc"][:])),
        ]

    def rest_thunks(sp, P, native_stt, lo, hi):
        X31A = stage[:, sp, lo:hi, :, 0]
        X31B = stage[:, sp, lo:hi, :, 1]
        Z32A = stage[:, sp, lo:hi, :, 2]
        Z32B = stage[:, sp, lo:hi, :, 3]
        n = hi - lo
        r = {}

        def rt(name):
            r[name] = qp.tile([128, n, 4], F32, name=f"r_{name}",
                              tag=f"r{n}_{name}", bufs=2)
            return r[name][:]

        R = lambda n_: r[n_][:]
        gsp = lambda n_: g[n_][:, sp, lo:hi, :]

        def stt(dst, a, s, b, op1):
            # dst = (a * s) op1 b; expanded for Pool (no stt support)
            if native_stt:
                return [lambda: P.scalar_tensor_tensor(
                    dst() if callable(dst) else dst, a(), s, b(),
                    op0=OP.mult, op1=op1)]
            nm = f"t{len(r)}"
            return [
                lambda: P.tensor_scalar(rt(nm), a(), s, None, op0=OP.mult),
                lambda: P.tensor_tensor(dst() if callable(dst) else dst,
                                        R(nm), b(), op=op1),
            ]

        th = [
            lambda: nc.scalar.activation(rt("s1"), Z32A, AF.Sigmoid),
            lambda: nc.scalar.activation(rt("s2"), Z32B, AF.Sigmoid),
            lambda: P.tensor_tensor(rt("r1"), R("s1"), R("s2"), op=OP.add),
            lambda: P.tensor_tensor(rt("r2"), R("s1"), R("s2"), op=OP.mult),
            lambda: P.tensor_tensor(rt("r3"), R("r1"), gsp("S"), op=OP.mult),
            lambda: P.tensor_tensor(rt("r4"), R("r2"), gsp("barq"),
                                    op=OP.mult),
        ]
        th += stt(lambda: rt("ha"), lambda: R("r3"), 20.0,
                  lambda: gsp("v2q"), OP.add)
        th += stt(lambda: rt("hb"), lambda: R("r4"), 200.0,
                  lambda: R("ha"), OP.add)
        th += stt(lambda: rt("hq"), lambda: R("r2"), -72.0,
                  lambda: R("hb"), OP.add)
        th += [
            lambda: P.tensor_tensor(rt("n1"), gsp("G1q"), X31A, op=OP.mult),
            lambda: P.tensor_tensor(rt("n2"), gsp("G2q"), X31B, op=OP.mult),
        ]
        th += stt(lambda: rt("n3"), lambda: R("n1"), -2.0,
                  lambda: R("n2"), OP.add)
        th += stt(lambda: rt("n4"), lambda: R("hq"), -0.4,
                  lambda: R("n3"), OP.add)
        th += [
            lambda: P.tensor_tensor(rt("l1"), R("n4"), gsp("rec"),
                                    op=OP.mult),
            lambda: P.tensor_scalar(rt("l2"), R("l1"), 0.0, None, op0=OP.max),
            lambda: P.tensor_tensor(rt("w1"), R("l2"), gsp("G1q"),
                                    op=OP.mult),
        ]
        th += stt(u_sb[:, sp, lo:hi, :, 0], lambda: R("w1"), -0.5,
                  lambda: X31A, OP.subtract)
        th += [
            lambda: P.tensor_tensor(rt("w2"), R("l2"), gsp("G2q"),
                                    op=OP.mult),
        ]
        th += stt(u_sb[:, sp, lo:hi, :, 1], lambda: R("w2"), 0.25,
                  lambda: X31B, OP.subtract)
        th += [lambda: nc.sync.dma_start(u_out[:, sp, lo:hi],
                                         u_sb[:, sp, lo:hi])]
        return th

    geo = geo_thunks()
    # QP rest pieces: ready-window -> thunk list. All overlapped on Pool
    # except the very last piece, which runs on (then-idle) DVE at the tail.
    pieces = []
    for sp in range(NSP):
        for (lo, hi) in ((0, 16), (16, 24), (24, 32)):
            last = (sp == NSP - 1 and lo == 24)
            eng = nc.vector if last else nc.gpsimd
            ready = 32 * sp + hi + 1
            pieces.append([ready, rest_thunks(sp, eng, last, lo, hi)])

    h1s = {}
    h2s = {}
    p3t = None

    SCL1 = float(2.0 ** -5)   # psum 256*z1 -> h1f8 = 8*h1
    SCL2 = float(2.0 ** -7)   # psum 128*(z2+b2) -> h2 = relu(z2+b2)
    CS = 452                  # c3 col split: Pool [0:CS], ACT [CS:512]

    for w in range(T + 3):
        # QP geo first in each window so its ops never head-of-line block
        # tile work queued after them (deps are >=1 window old by then)
        if 7 <= w and geo:
            geo.pop(0)()
        if w < T:
            t = w
            # c2/c3 first: pairB feeds the DVE merged relu, which should
            # start as early as possible in the window
            pairB = pairs.tile([128, 2, 512], F32, name="pairB", tag="pB",
                               bufs=1)
            pairA = pairs.tile([128, 2, 512], F32, name="pairA", tag="pA",
                               bufs=1)
            for c in (2, 3, 0, 1):
                dst = (pairA if c < 2 else pairB)[:, c % 2, :]
                nc.tensor.matmul(dst, sb_w1[:, :, c, :], sb_xdr[:, t, :, :],
                                 start=True, stop=True, perf_mode=DRM)
            h1A = hp.tile([128, 2, 512], F8, name="h1A", tag="h1", bufs=4)
            h1B = hp.tile([128, 2, 512], F8, name="h1B", tag="h1", bufs=4)
            nc.vector.tensor_scalar(h1B[:], pairB[:], SCL1, 0.0,
                                    op0=OP.mult, op1=OP.max)
            nc.scalar.activation(h1A[:], pairA[:], AF.Relu, scale=SCL1)
            h1s[t] = (h1A, h1B)
        if 0 <= w - 1 < T:
            t = w - 1
            h1A, h1B = h1s.pop(t)
            h2t = hp.tile([128, 2, 512], F32R, name="h2t", tag="h2", bufs=2)
            # L2 in sample-halves: each half's psum is ONE bank [128,2,256]
            # (h21 plane 0, h22 plane 1), so depth-2 costs only 2 banks
            for u, eng in ((0, "act"), (1, "dve")):
                lo, hi = 256 * u, 256 * (u + 1)
                pH = pairs.tile([128, 2, 256], F32, name=f"pH{u}", tag="pH",
                                bufs=2)
                for half in range(2):
                    bank = pH[:, half, :]
                    nc.tensor.matmul(bank, sb_w2[:, :, half, 0, :],
                                     h1A[:, :, lo:hi],
                                     start=True, stop=False, perf_mode=DRM)
                    nc.tensor.matmul(bank, sb_w2[:, :, half, 1, :],
                                     h1B[:, :, lo:hi],
                                     start=False, stop=False, perf_mode=DRM)
                    nc.tensor.matmul(bank, sb_b2[:, :, half, :],
                                     sb_ones8[:, :, lo:hi],
                                     start=False, stop=True, perf_mode=DRM)
                if eng == "act":
                    nc.scalar.activation(h2t[:, :, lo:hi], pH[:], AF.Relu,
                                         scale=SCL2)
                else:
                    nc.vector.tensor_scalar(h2t[:, :, lo:hi], pH[:], SCL2,
                                            0.0, op0=OP.mult, op1=OP.max)
            h2s[t] = h2t
        if 0 <= w - 2 < T:
            t = w - 2
            g32 = t % 32
            sp = t // 32
            if g32 == 0:
                p3t = p3p.tile([128, 32, 4, 4], F32, name="p3t", tag="p3",
                               bufs=2)
            h2t = h2s.pop(t)
            for j in range(4):
                out = p3t[:, g32, j, :]
                nc.tensor.matmul(out, h2t[:, 0, 128 * j:128 * (j + 1)],
                                 sb_w3[:, 0, :],
                                 start=(g32 == 0 and j == 0), stop=False)
                nc.tensor.matmul(out, h2t[:, 1, 128 * j:128 * (j + 1)],
                                 sb_w3[:, 1, :], start=False, stop=False)
                nc.tensor.matmul(out, sb_onescol[:], sb_b3row[:],
                                 start=False, stop=(g32 == 31 and j == 3))
            if g32 in (15, 23, 31):
                lo = {15: 0, 23: 16, 31: 24}[g32]
                # psum->sbuf copy on ACT (Copy activation)
                nc.scalar.activation(stage[:, sp, lo:g32 + 1],
                                     p3t[:, lo:g32 + 1], AF.Copy)
        for pc in pieces:
            if pc[0] <= w and not geo and pc[1]:
                for _ in range(5):
                    if pc[1]:
                        pc[1].pop(0)()
                break
    while geo:
        geo.pop(0)()
    for pc in pieces:
        while pc[1]:
            pc[1].pop(0)()


def _build_kernel(n_cores, B):
    nc = bacc_mod.Bacc("TRN2", target_bir_lowering=False, debug=False,
                       num_devices=n_cores)
    T = B // 512
    NSP = T // 32
    Fl = B // 128
    xdr = nc.dram_tensor("xdr", [10, B // 512, 2, 512], F8,
                         kind="ExternalInput").ap()
    xq = nc.dram_tensor("xq", [128, 4, Fl], F32, kind="ExternalInput").ap()
    w1dr = nc.dram_tensor("w1dr", [10, 2, 4, 128], F8,
                          kind="ExternalInput").ap()
    w2dr = nc.dram_tensor("w2dr", [128, 2, 2, 2, 128], F8,
                          kind="ExternalInput").ap()
    b2dr = nc.dram_tensor("b2dr", [1, 2, 2, 128], F8,
                          kind="ExternalInput").ap()
    b2f32 = nc.dram_tensor("b2f32", [128, 2], F32, kind="ExternalInput").ap()
    ones8 = nc.dram_tensor("ones8", [1, 2, 512], F8,
                           kind="ExternalInput").ap()
    w3d = nc.dram_tensor("w3d", [128, 2, 4], F32R, kind="ExternalInput").ap()
    onescol = nc.dram_tensor("onescol", [1, 128], F32R,
                             kind="ExternalInput").ap()
    b3row = nc.dram_tensor("b3row", [1, 4], F32R, kind="ExternalInput").ap()
    u_out = nc.dram_tensor("u_out", [128, NSP, 32, 4, 2], F32,
                           kind="ExternalOutput").ap()
    aps = (xdr, xq, w1dr, w2dr, b2dr, b2f32, ones8, w3d, onescol, b3row,
           u_out)
    with tile.TileContext(nc) as tc:
        with ExitStack() as ctx:
            _emit(nc, tc, ctx, aps, B)
    nc.compile()
    return nc


def _e4(a):
    return np.clip(np.asarray(a, np.float32), -240.0, 240.0).astype(E4)


def _prep_core_inputs(x_shard, W1, b1, W21, b21, W22, b22, W31, b31, W32,
                      b32):
    Bc = x_shard.shape[0]
    Tc = Bc // 512
    x_shard = np.ascontiguousarray(x_shard, dtype=np.float32)

    # xdr: [10, 2, B] fp8; tile t cols j*128+p = sample p*256+4t+j
    xs = x_shard.reshape(128, Tc, 4, 4)
    xT4 = np.ascontiguousarray(xs.transpose(3, 1, 2, 0)).reshape(4, Bc)
    xe5 = np.concatenate([xT4, np.ones((1, Bc), np.float32)], axis=0)
    Xh = _e4(16.0 * xe5)
    Xl = _e4(16.0 * xe5 - Xh.astype(np.float32))
    xdr2 = np.empty((10, 2, Bc), dtype=E4)
    xdr2[0:5, 0] = Xh
    xdr2[0:5, 1] = Xl
    xdr2[5:10, 0] = Xh
    xdr2[5:10, 1] = Xl
    # [k, i, t*512+n] -> [k, t, i, n]: DoubleRow plane stride must be small
    xdr = np.ascontiguousarray(
        xdr2.reshape(10, 2, Tc, 512).transpose(0, 2, 1, 3))

    # w1dr: [10, 2, 4, 128]
    W1e = np.concatenate([W1.T, b1[None, :]], axis=0)  # [5, 512]
    Wh = _e4(16.0 * W1e)
    Wl = _e4(16.0 * W1e - Wh.astype(np.float32))
    w1dr = np.empty((10, 2, 4, 128), dtype=E4)
    w1dr[0:5, 0] = Wh.reshape(5, 4, 128)
    w1dr[0:5, 1] = Wh.reshape(5, 4, 128)
    w1dr[5:10, 0] = Wl.reshape(5, 4, 128)
    w1dr[5:10, 1] = Wl.reshape(5, 4, 128)

    # w2dr: [k, i, half, q, m] = e4(16*W2h)[m, 256q+128i+k]
    w2dr = np.empty((128, 2, 2, 2, 128), dtype=E4)
    for half, W2h in ((0, W21), (1, W22)):
        q16 = _e4(16.0 * W2h)  # [128(m), 512(n)]
        w2dr[:, :, half, :, :] = q16.reshape(128, 2, 2, 128).transpose(
            3, 2, 1, 0)

    # b2dr hi/lo planes: psum scale 128*b2
    b2dr = np.empty((1, 2, 2, 128), dtype=E4)
    for half, b2h in ((0, b21), (1, b22)):
        hi = _e4(128.0 * b2h)
        lo = _e4(128.0 * b2h - hi.astype(np.float32))
        b2dr[0, 0, half] = hi
        b2dr[0, 1, half] = lo

    ones8 = np.ones((1, 2, 512), dtype=E4)
    b2f32 = np.stack([b21, b22], axis=1).astype(np.float32)

    w3d = np.zeros((128, 2, 4), dtype=np.float32)
    w3d[:, 0, 0:2] = W31.T
    w3d[:, 1, 2:4] = W32.T
    onescol = np.ones((1, 128), dtype=np.float32)
    b3row = np.concatenate([b31, b32]).astype(np.float32).reshape(1, 4)

    xq = np.ascontiguousarray(
        x_shard.reshape(128, Bc // 128, 4).transpose(0, 2, 1))

    return {
        "xdr": xdr, "xq": xq, "w1dr": w1dr, "w2dr": w2dr, "b2dr": b2dr,
        "b2f32": b2f32, "ones8": ones8, "w3d": w3d, "onescol": onescol,
        "b3row": b3row,
    }


def kernel(x, W1, b1, W21, b21, W22, b22, W31, b31, W32, b32, sgn=None):
    x = np.asarray(x, dtype=np.float32)
    args = [np.asarray(a, dtype=np.float32)
            for a in (W1, b1, W21, b21, W22, b22, W31, b31, W32, b32)]

    if "nc" not in _CACHE:
        _CACHE["nc"] = _build_kernel(N_CORES, B)
    nc = _CACHE["nc"]

    in_maps = []
    for c in range(N_CORES):
        shard = x[c * B:(c + 1) * B]
        in_maps.append(_prep_core_inputs(shard, *args))

    res = run_bass_kernel_spmd(nc, in_maps, core_ids=list(range(N_CORES)))
    out = np.empty((NB, 2), dtype=np.float32)
    for c in range(N_CORES):
        out[c * B:(c + 1) * B] = res.results[c]["u_out"].reshape(B, 2)
    return out


# revision 26
# speedup vs baseline: 1.2119x; 1.2119x over previous
"""BarrierNet (MLP 4->512->{128,128}->{2,2} + closed-form QP) on 8 Trainium2 cores.

Data-parallel: batch 262144 sharded 8 x 32768; weights replicated.

v2 design (cost-model driven):
- L1 runs as fp8e4m3 DoubleRow matmuls (0.5 cyc/row): the 4(+bias) input
  features are expanded host-side into a 4-term hi/lo fp8 split (20
  contraction items packed as K=10 x 2 planes), so L1 keeps ~bf16 accuracy
  at half the f32r streaming cost.
- L2 contracts the 512 h1 neurons as fp8 DoubleRow (2 matmuls of K=256 per
  output half) plus a 1-pair DoubleRow bias matmul (hi/lo planes), so the
  psum holds relu-ready pre-activations and the vector pass is a pure
  scale+relu+cast.
- L3 is transposed: lhsT = h2 (stationary), rhs = tiny W3 column block, so
  outputs land sample-major [128, 4] in psum at ~7ns/matmul and the QP phase
  needs no transposes, DRAM bounces, or staging matmuls.
- The three psum->sbuf passes per tile (h1 relu x2, h2 relu) are split
  across ACT / Pool / DVE to balance engine time; the QP chain runs wide
  [128, 256] with scalar_tensor_tensor fusion, ACT Square/Sin/Abs offload,
  and sample-major psum results copied straight into SBUF.
"""
import numpy as np
import ml_dtypes
from contextlib import ExitStack

import concourse.bass as bass
from concourse import bacc as bacc_mod
import concourse.tile as tile
from concourse import mybir
from concourse.bass_utils import run_bass_kernel_spmd

F32 = mybir.dt.float32
F32R = mybir.dt.float32r
F8 = mybir.dt.float8e4
E4 = ml_dtypes.float8_e4m3
AF = mybir.ActivationFunctionType
OP = mybir.AluOpType
DRM = mybir.MatmulPerfMode.DoubleRow

MAGIC = float(np.float32(1.5 * 2 ** 23))
INV2PI = float(np.float32(1.0 / (2 * np.pi)))
TWOPI = float(np.float32(2 * np.pi))
HALFPI = float(np.float32(np.pi / 2))
GG_EPS = float(np.float32(1e-12 / 1600.0))

N_CORES = 8
NB = 262144
B = NB // N_CORES      # 32768 per core
T = B // 512           # 64 tiles of 512 samples
F = B // 128           # 256 samples per partition
NSP = T // 32          # 2 sp groups (32 tiles / 128 f's each)
CB = 448               # reluB col split: Pool [0:CB], ACT [CB:512]

_CACHE = {}


def _emit(nc, tc, ctx, aps, B):
    (xdr, xq, w1dr, w2dr, b2dr, b2f32, ones8, w3d, onescol, b3row,
     u_out) = aps
    T = B // 512
    NSP = T // 32

    const = ctx.enter_context(tc.tile_pool(name="const", bufs=1))
    hp = ctx.enter_context(tc.tile_pool(name="hp", bufs=1))
    qp = ctx.enter_context(tc.tile_pool(name="qp", bufs=1))
    pairs = ctx.enter_context(tc.tile_pool(name="pairs", bufs=1, space="PSUM"))
    p3p = ctx.enter_context(tc.tile_pool(name="p3p", bufs=1, space="PSUM"))

    # ---- constant staging (order = DMA issue order) ----
    # xdr split in 4 so tile 0 can start after the first quarter lands
    sb_xdr = const.tile([10, B // 512, 2, 512], F8, name="sb_xdr",
                        tag="sb_xdr")
    TT = B // 512
    XCH = [0, 4, 16, TT // 2, TT]
    nc.sync.dma_start(sb_xdr[:, 0:4], xdr[:, 0:4])
    sb_w1 = const.tile([10, 2, 4, 128], F8, name="sb_w1", tag="sb_w1")
    nc.sync.dma_start(sb_w1[:], w1dr[:])
    nc.sync.dma_start(sb_xdr[:, XCH[1]:XCH[2]], xdr[:, XCH[1]:XCH[2]])
    sb_w2 = const.tile([128, 2, 2, 2, 128], F8, name="sb_w2", tag="sb_w2")
    nc.sync.dma_start(sb_w2[:], w2dr[:])
    sb_b2 = const.tile([1, 2, 2, 128], F8, name="sb_b2", tag="sb_b2")
    nc.sync.dma_start(sb_b2[:], b2dr[:])
    sb_ones8 = const.tile([1, 2, 512], F8, name="sb_ones8", tag="sb_ones8")
    nc.sync.dma_start(sb_ones8[:], ones8[:])
    sb_w3 = const.tile([128, 2, 4], F32R, name="sb_w3", tag="sb_w3")
    nc.sync.dma_start(sb_w3[:], w3d[:])
    sb_onescol = const.tile([1, 128], F32R, name="sb_onescol", tag="sb_onescol")
    nc.sync.dma_start(sb_onescol[:], onescol[:])
    sb_b3row = const.tile([1, 4], F32R, name="sb_b3row", tag="sb_b3row")
    nc.sync.dma_start(sb_b3row[:], b3row[:])
    sb_xq = const.tile([128, 4, B // 128], F32, name="sb_xq", tag="sb_xq")
    nc.sync.dma_start(sb_xq[:], xq[:])
    for k in range(2, 4):
        nc.sync.dma_start(sb_xdr[:, XCH[k]:XCH[k + 1]],
                          xdr[:, XCH[k]:XCH[k + 1]])

    stage = const.tile([128, NSP, 32, 4, 4], F32, name="stage", tag="stage")
    u_sb = const.tile([128, NSP, 32, 4, 2], F32, name="u_sb", tag="u_sb")

    bias05 = const.tile([128, 1], F32, name="bias05", tag="bias05")
    nc.gpsimd.memset(bias05[:], 0.5)
    bias25 = const.tile([128, 1], F32, name="bias25", tag="bias25")
    nc.gpsimd.memset(bias25[:], 2.5)
    biasHP = const.tile([128, 1], F32, name="biasHP", tag="biasHP")
    nc.gpsimd.memset(biasHP[:], HALFPI)
    # warm every ACT table during the input-DMA shadow so no
    # LoadActFuncSet lands mid-pipeline
    warm = const.tile([128, 1], F32, name="warm", tag="warm")
    for fn in (AF.Relu, AF.Copy, AF.Abs, AF.Sin, AF.Sigmoid):
        nc.scalar.activation(warm[:], bias05[:], fn)

    # ---- QP program ----
    PX = sb_xq[:, 0, :].rearrange("p (s g j) -> p s g j", s=NSP, g=32, j=4)
    PY = sb_xq[:, 1, :].rearrange("p (s g j) -> p s g j", s=NSP, g=32, j=4)
    TH = sb_xq[:, 2, :].rearrange("p (s g j) -> p s g j", s=NSP, g=32, j=4)
    VV = sb_xq[:, 3, :].rearrange("p (s g j) -> p s g j", s=NSP, g=32, j=4)

    def gt(name):
        return qp.tile([128, NSP, 32, 4], F32, name=name, tag=name)

    g = {}

    def geo_thunks():
        # each thunk emits ONE instruction; order respects deps.
        # Pool (gpsimd) does all SBUF->SBUF tensor work (it cannot touch
        # PSUM), ACT does Sin/Abs, DVE only the reciprocal.
        def dve(name, fn):
            def run():
                g[name] = gt(name)
                fn(g[name][:])
            return run

        def mk_ts(name, src, s1, op0, s2=None, op1=None):
            def emit(o):
                a = src() if callable(src) else src
                if op1 is None:
                    nc.gpsimd.tensor_scalar(o, a, s1, None, op0=op0)
                else:
                    nc.gpsimd.tensor_scalar(o, a, s1, s2, op0=op0, op1=op1)
            return dve(name, emit)

        def mk_tt(name, a, b, op):
            return dve(name, lambda o: nc.gpsimd.tensor_tensor(
                o, a(), b(), op=op))

        def mk_act(name, a, func, scale=1.0, bias=0.0):
            return dve(name, lambda o: nc.scalar.activation(
                o, a(), func, scale=scale, bias=bias))

        G = lambda n: (lambda: g[n][:])
        return [
            mk_ts("dxp", lambda: PX, 1.0, OP.add),
            mk_ts("dyp", lambda: PY, 0.5, OP.add),
            mk_ts("vp", lambda: VV, 2.5, OP.add),
            mk_ts("f1", lambda: TH, INV2PI, OP.mult, MAGIC, OP.add),
            mk_ts("f2", G("f1"), MAGIC, OP.subtract, TWOPI, OP.mult),
            mk_tt("thr", lambda: TH, G("f2"), OP.subtract),
            mk_act("st", G("thr"), AF.Sin),
            mk_act("ab", G("thr"), AF.Abs),
            mk_act("ct", G("ab"), AF.Sin, scale=-1.0, bias=biasHP[:]),
            mk_tt("vst2", G("vp"), G("st"), OP.mult),
            mk_tt("vct2", G("vp"), G("ct"), OP.mult),
            mk_tt("ga", G("dxp"), G("vst2"), OP.mult),
            mk_tt("gb", G("dyp"), G("vct2"), OP.mult),
            mk_tt("G1q", G("ga"), G("gb"), OP.subtract),
            mk_tt("gc", G("dxp"), G("vct2"), OP.mult),
            mk_tt("gd", G("dyp"), G("vst2"), OP.mult),
            mk_tt("S", G("gc"), G("gd"), OP.add),
            mk_tt("ge", G("dxp"), G("ct"), OP.mult),
            mk_tt("gf", G("dyp"), G("st"), OP.mult),
            mk_tt("G2q", G("ge"), G("gf"), OP.add),
            mk_tt("q1", G("G1q"), G("G1q"), OP.mult),
            mk_tt("q2t", G("G2q"), G("G2q"), OP.mult),
            mk_ts("q2", G("q2t"), 0.25, OP.mult),
            mk_tt("q3", G("q1"), G("q2"), OP.add),
            mk_ts("ggc", G("q3"), GG_EPS, OP.max),
            mk_tt("dx2", G("dxp"), G("dxp"), OP.mult),
            mk_tt("dy2", G("dyp"), G("dyp"), OP.mult),
            mk_tt("barq", G("dx2"), G("dy2"), OP.add),
            mk_tt("v2q", G("vp"), G("vp"), OP.mult),
            dve("rec", lambda o: nc.vector.reciprocal(o, g["ggc"][:])),
        ]

    def rest_thunks(sp, P, native_stt, lo, hi):
        X31A = stage[:, sp, lo:hi, :, 0]
        X31B = stage[:, sp, lo:hi, :, 1]
        Z32A = stage[:, sp, lo:hi, :, 2]
        Z32B = stage[:, sp, lo:hi, :, 3]
        n = hi - lo
        r = {}

        def rt(name):
            r[name] = qp.tile([128, n, 4], F32, name=f"r_{name}",
                              tag=f"r{n}_{name}", bufs=2)
            return r[name][:]

        R = lambda n_: r[n_][:]
        gsp = lambda n_: g[n_][:, sp, lo:hi, :]

        def stt(dst, a, s, b, op1):
            # dst = (a * s) op1 b; expanded for Pool (no stt support)
            if native_stt:
                return [lambda: P.scalar_tensor_tensor(
                    dst() if callable(dst) else dst, a(), s, b(),
                    op0=OP.mult, op1=op1)]
            nm = f"t{len(r)}"
            return [
                lambda: P.tensor_scalar(rt(nm), a(), s, None, op0=OP.mult),
                lambda: P.tensor_tensor(dst() if callable(dst) else dst,
                                        R(nm), b(), op=op1),
            ]

        th = [
            lambda: nc.scalar.activation(rt("s1"), Z32A, AF.Sigmoid),
            lambda: nc.scalar.activation(rt("s2"), Z32B, AF.Sigmoid),
            lambda: P.tensor_tensor(rt("r1"), R("s1"), R("s2"), op=OP.add),
            lambda: P.tensor_tensor(rt("r2"), R("s1"), R("s2"), op=OP.mult),
            lambda: P.tensor_tensor(rt("r3"), R("r1"), gsp("S"), op=OP.mult),
            lambda: P.tensor_tensor(rt("r4"), R("r2"), gsp("barq"),
                                    op=OP.mult),
        ]
        th += stt(lambda: rt("ha"), lambda: R("r3"), 20.0,
                  lambda: gsp("v2q"), OP.add)
        th += stt(lambda: rt("hb"), lambda: R("r4"), 200.0,
                  lambda: R("ha"), OP.add)
        th += stt(lambda: rt("hq"), lambda: R("r2"), -72.0,
                  lambda: R("hb"), OP.add)
        th += [
            lambda: P.tensor_tensor(rt("n1"), gsp("G1q"), X31A, op=OP.mult),
            lambda: P.tensor_tensor(rt("n2"), gsp("G2q"), X31B, op=OP.mult),
        ]
        th += stt(lambda: rt("n3"), lambda: R("n1"), -2.0,
                  lambda: R("n2"), OP.add)
        th += stt(lambda: rt("n4"), lambda: R("hq"), -0.4,
                  lambda: R("n3"), OP.add)
        th += [
            lambda: P.tensor_tensor(rt("l1"), R("n4"), gsp("rec"),
                                    op=OP.mult),
            lambda: P.tensor_scalar(rt("l2"), R("l1"), 0.0, None, op0=OP.max),
            lambda: P.tensor_tensor(rt("w1"), R("l2"), gsp("G1q"),
                                    op=OP.mult),
        ]
        th += stt(u_sb[:, sp, lo:hi, :, 0], lambda: R("w1"), -0.5,
                  lambda: X31A, OP.subtract)
        th += [
            lambda: P.tensor_tensor(rt("w2"), R("l2"), gsp("G2q"),
                                    op=OP.mult),
        ]
        th += stt(u_sb[:, sp, lo:hi, :, 1], lambda: R("w2"), 0.25,
                  lambda: X31B, OP.subtract)
        th += [lambda: nc.sync.dma_start(u_out[:, sp, lo:hi],
                                         u_sb[:, sp, lo:hi])]
        return th

    geo = geo_thunks()
    # QP rest pieces: ready-window -> thunk list. All overlapped on Pool
    # except the very last piece, which runs on (then-idle) DVE at the tail.
    pieces = []
    for sp in range(NSP):
        for (lo, hi) in ((0, 16), (16, 24), (24, 32)):
            last = (sp == NSP - 1 and lo == 24)
            eng = nc.vector if last else nc.gpsimd
            ready = 32 * sp + hi + 1
            pieces.append([ready, rest_thunks(sp, eng, last, lo, hi)])

    h1s = {}
    h2s = {}
    p3t = None

    SCL1 = float(2.0 ** -5)   # psum 256*z1 -> h1f8 = 8*h1
    SCL2 = float(2.0 ** -7)   # psum 128*(z2+b2) -> h2 = relu(z2+b2)
    CS = 452                  # c3 col split: Pool [0:CS], ACT [CS:512]

    for w in range(T + 3):
        # QP geo first in each window so its ops never head-of-line block
        # tile work queued after them (deps are >=1 window old by then)
        if 7 <= w and geo:
            geo.pop(0)()
        if w < T:
            t = w
            # c2/c3 first: pairB feeds the DVE merged relu, which should
            # start as early as possible in the window
            pairB = pairs.tile([128, 2, 512], F32, name="pairB", tag="pB",
                               bufs=1)
            pairA = pairs.tile([128, 2, 512], F32, name="pairA", tag="pA",
                               bufs=1)
            for c in (2, 3, 0, 1):
                dst = (pairA if c < 2 else pairB)[:, c % 2, :]
                nc.tensor.matmul(dst, sb_w1[:, :, c, :], sb_xdr[:, t, :, :],
                                 start=True, stop=True, perf_mode=DRM)
            h1A = hp.tile([128, 2, 512], F8, name="h1A", tag="h1", bufs=4)
            h1B = hp.tile([128, 2, 512], F8, name="h1B", tag="h1", bufs=4)
            nc.scalar.activation(h1B[:], pairB[:], AF.Relu, scale=SCL1)
            nc.vector.tensor_scalar(h1A[:], pairA[:], SCL1, 0.0,
                                    op0=OP.mult, op1=OP.max)
            h1s[t] = (h1A, h1B)
        if 0 <= w - 1 < T:
            t = w - 1
            h1A, h1B = h1s.pop(t)
            h2t = hp.tile([128, 2, 512], F32R, name="h2t", tag="h2", bufs=2)
            # L2 in sample-halves: each half's psum is ONE bank [128,2,256]
            # (h21 plane 0, h22 plane 1), so depth-2 costs only 2 banks
            for u, eng in ((0, "act"), (1, "dve")):
                lo, hi = 256 * u, 256 * (u + 1)
                pH = pairs.tile([128, 2, 256], F32, name=f"pH{u}", tag="pH",
                                bufs=2)
                for half in range(2):
                    bank = pH[:, half, :]
                    nc.tensor.matmul(bank, sb_w2[:, :, half, 0, :],
                                     h1A[:, :, lo:hi],
                                     start=True, stop=False, perf_mode=DRM)
                    nc.tensor.matmul(bank, sb_w2[:, :, half, 1, :],
                                     h1B[:, :, lo:hi],
                                     start=False, stop=False, perf_mode=DRM)
                    nc.tensor.matmul(bank, sb_b2[:, :, half, :],
                                     sb_ones8[:, :, lo:hi],
                                     start=False, stop=True, perf_mode=DRM)
                if eng == "act":
                    nc.scalar.activation(h2t[:, :, lo:hi], pH[:], AF.Relu,
                                         scale=SCL2)
                else:
                    nc.vector.tensor_scalar(h2t[:, :, lo:hi], pH[:], SCL2,
                                            0.0, op0=OP.mult, op1=OP.max)
            h2s[t] = h2t
        if 0 <= w - 2 < T:
            t = w - 2
            g32 = t % 32
            sp = t // 32
            if g32 == 0:
                p3t = p3p.tile([128, 32, 4, 4], F32, name="p3t", tag="p3",
                               bufs=2)
            h2t = h2s.pop(t)
            for j in range(4):
                out = p3t[:, g32, j, :]
                nc.tensor.matmul(out, h2t[:, 0, 128 * j:128 * (j + 1)],
                                 sb_w3[:, 0, :],
                                 start=(g32 == 0 and j == 0), stop=False)
                nc.tensor.matmul(out, h2t[:, 1, 128 * j:128 * (j + 1)],
                                 sb_w3[:, 1, :], start=False, stop=False)
                nc.tensor.matmul(out, sb_onescol[:], sb_b3row[:],
                                 start=False, stop=(g32 == 31 and j == 3))
            if g32 in (15, 23, 31):
                lo = {15: 0, 23: 16, 31: 24}[g32]
                # psum->sbuf copy on ACT (Copy activation)
                nc.scalar.activation(stage[:, sp, lo:g32 + 1],
                                     p3t[:, lo:g32 + 1], AF.Copy)
        for pc in pieces:
            if pc[0] <= w and not geo and pc[1]:
                for _ in range(5):
                    if pc[1]:
                        pc[1].pop(0)()
                break
    while geo:
        geo.pop(0)()
    for pc in pieces:
        while pc[1]:
            pc[1].pop(0)()


def _build_kernel(n_cores, B):
    nc = bacc_mod.Bacc("TRN2", target_bir_lowering=False, debug=False,
                       num_devices=n_cores)
    T = B // 512
    NSP = T // 32
    Fl = B // 128
    xdr = nc.dram_tensor("xdr", [10, B // 512, 2, 512], F8,
                         kind="ExternalInput").ap()
    xq = nc.dram_tensor("xq", [128, 4, Fl], F32, kind="ExternalInput").ap()
    w1dr = nc.dram_tensor("w1dr", [10, 2, 4, 128], F8,
                          kind="ExternalInput").ap()
    w2dr = nc.dram_tensor("w2dr", [128, 2, 2, 2, 128], F8,
                          kind="ExternalInput").ap()
    b2dr = nc.dram_tensor("b2dr", [1, 2, 2, 128], F8,
                          kind="ExternalInput").ap()
    b2f32 = nc.dram_tensor("b2f32", [128, 2], F32, kind="ExternalInput").ap()
    ones8 = nc.dram_tensor("ones8", [1, 2, 512], F8,
                           kind="ExternalInput").ap()
    w3d = nc.dram_tensor("w3d", [128, 2, 4], F32R, kind="ExternalInput").ap()
    onescol = nc.dram_tensor("onescol", [1, 128], F32R,
                             kind="ExternalInput").ap()
    b3row = nc.dram_tensor("b3row", [1, 4], F32R, kind="ExternalInput").ap()
    u_out = nc.dram_tensor("u_out", [128, NSP, 32, 4, 2], F32,
                           kind="ExternalOutput").ap()
    aps = (xdr, xq, w1dr, w2dr, b2dr, b2f32, ones8, w3d, onescol, b3row,
           u_out)
    with tile.TileContext(nc) as tc:
        with ExitStack() as ctx:
            _emit(nc, tc, ctx, aps, B)
    nc.compile()
    return nc


def _e4(a):
    return np.clip(np.asarray(a, np.float32), -240.0, 240.0).astype(E4)


def _prep_core_inputs(x_shard, W1, b1, W21, b21, W22, b22, W31, b31, W32,
                      b32):
    Bc = x_shard.shape[0]
    Tc = Bc // 512
    x_shard = np.ascontiguousarray(x_shard, dtype=np.float32)

    # xdr: [10, 2, B] fp8; tile t cols j*128+p = sample p*256+4t+j
    xs = x_shard.reshape(128, Tc, 4, 4)
    xT4 = np.ascontiguousarray(xs.transpose(3, 1, 2, 0)).reshape(4, Bc)
    xe5 = np.concatenate([xT4, np.ones((1, Bc), np.float32)], axis=0)
    Xh = _e4(16.0 * xe5)
    Xl = _e4(16.0 * xe5 - Xh.astype(np.float32))
    xdr2 = np.empty((10, 2, Bc), dtype=E4)
    xdr2[0:5, 0] = Xh
    xdr2[0:5, 1] = Xl
    xdr2[5:10, 0] = Xh
    xdr2[5:10, 1] = Xl
    # [k, i, t*512+n] -> [k, t, i, n]: DoubleRow plane stride must be small
    xdr = np.ascontiguousarray(
        xdr2.reshape(10, 2, Tc, 512).transpose(0, 2, 1, 3))

    # w1dr: [10, 2, 4, 128]
    W1e = np.concatenate([W1.T, b1[None, :]], axis=0)  # [5, 512]
    Wh = _e4(16.0 * W1e)
    Wl = _e4(16.0 * W1e - Wh.astype(np.float32))
    w1dr = np.empty((10, 2, 4, 128), dtype=E4)
    w1dr[0:5, 0] = Wh.reshape(5, 4, 128)
    w1dr[0:5, 1] = Wh.reshape(5, 4, 128)
    w1dr[5:10, 0] = Wl.reshape(5, 4, 128)
    w1dr[5:10, 1] = Wl.reshape(5, 4, 128)

    # w2dr: [k, i, half, q, m] = e4(16*W2h)[m, 256q+128i+k]
    w2dr = np.empty((128, 2, 2, 2, 128), dtype=E4)
    for half, W2h in ((0, W21), (1, W22)):
        q16 = _e4(16.0 * W2h)  # [128(m), 512(n)]
        w2dr[:, :, half, :, :] = q16.reshape(128, 2, 2, 128).transpose(
            3, 2, 1, 0)

    # b2dr hi/lo planes: psum scale 128*b2
    b2dr = np.empty((1, 2, 2, 128), dtype=E4)
    for half, b2h in ((0, b21), (1, b22)):
        hi = _e4(128.0 * b2h)
        lo = _e4(128.0 * b2h - hi.astype(np.float32))
        b2dr[0, 0, half] = hi
        b2dr[0, 1, half] = lo

    ones8 = np.ones((1, 2, 512), dtype=E4)
    b2f32 = np.stack([b21, b22], axis=1).astype(np.float32)

    w3d = np.zeros((128, 2, 4), dtype=np.float32)
    w3d[:, 0, 0:2] = W31.T
    w3d[:, 1, 2:4] = W32.T
    onescol = np.ones((1, 128), dtype=np.float32)
    b3row = np.concatenate([b31, b32]).astype(np.float32).reshape(1, 4)

    xq = np.ascontiguousarray(
        x_shard.reshape(128, Bc // 128, 4).transpose(0, 2, 1))

    return {
        "xdr": xdr, "xq": xq, "w1dr": w1dr, "w2dr": w2dr, "b2dr": b2dr,
        "b2f32": b2f32, "ones8": ones8, "w3d": w3d, "onescol": onescol,
        "b3row": b3row,
    }


def kernel(x, W1, b1, W21, b21, W22, b22, W31, b31, W32, b32, sgn=None):
    x = np.asarray(x, dtype=np.float32)
    args = [np.asarray(a, dtype=np.float32)
            for a in (W1, b1, W21, b21, W22, b22, W31, b31, W32, b32)]

    if "nc" not in _CACHE:
        _CACHE["nc"] = _build_kernel(N_CORES, B)
    nc = _CACHE["nc"]

    in_maps = []
    for c in range(N_CORES):
        shard = x[c * B:(c + 1) * B]
        in_maps.append(_prep_core_inputs(shard, *args))

    res = run_bass_kernel_spmd(nc, in_maps, core_ids=list(range(N_CORES)))
    out = np.empty((NB, 2), dtype=np.float32)
    for c in range(N_CORES):
        out[c * B:(c + 1) * B] = res.results[c]["u_out"].reshape(B, 2)
    return out
